# revision 1
# baseline (speedup 1.0000x reference)
"""Trainium2 Bass kernel for nn_CAConvV2 (grouped causal conv + per-tap
feature roll + time mask, output (F, T, L*M, K)).

Self-contained: hardcodes shapes/sharding for
  x: (4, 1024, 512) f32, conv_w: (12288, 1, 3) f32, conv_b: (12288,) f32
  output: (512, 1024, 12, 8) f32

Sharding: 8 cores = 4 feature chunks (128) x 2 time halves (512).
No cross-core communication.
"""

import numpy as np

M, T, F = 4, 1024, 512
K, L, CK = 8, 3, 3
NCORES = 8
PCHUNK = 128  # features per core
THALF = 512   # time steps per core
TC = 256      # staging time chunk (legacy)
TCHUNKS = (192, 192, 96, 32)  # staging chunk sizes (sum = THALF)
HALO = 9      # max feature roll shift (K-1 + L-1)

_prog_cache = {}


def _build_program(timing=False):
    from concourse import mybir, bacc
    from concourse.tile import TileContext

    nc = bacc.Bacc("TRN2", target_bir_lowering=False, debug=False,
                   num_devices=NCORES)
    x_local = nc.dram_tensor("x_local", (HALO + 1, PCHUNK, M, THALF + 2),
                             mybir.dt.float16, kind="ExternalInput")
    # wpack columns: [w0 (24) | w1 (24) | w2 (24) | bias (24)]
    wpack = nc.dram_tensor("wpack", (PCHUNK, 96), mybir.dt.float32,
                           kind="ExternalInput")
    out_local = nc.dram_tensor("out_local", (PCHUNK, THALF * 96),
                               mybir.dt.float16,
                               kind="Internal" if timing else "ExternalOutput")
    if timing:
        marker = nc.dram_tensor("marker", (PCHUNK, 1), mybir.dt.float32,
                                kind="ExternalOutput")

    # (i, l) pairs ordered by shift s = i + l so compute can start as soon as
    # the first shifted x windows arrive.
    IL = sorted(((i, l) for i in range(K) for l in range(L)),
                key=lambda p: (p[0] + p[1], p[1]))


    with TileContext(nc) as tc:
        with tc.tile_pool(name="xp", bufs=1) as xpool, \
             tc.tile_pool(name="wp", bufs=1) as wpool, \
             tc.tile_pool(name="work", bufs=12) as work, \
             tc.tile_pool(name="stg", bufs=2) as stg:
            wt = wpool.tile([PCHUNK, 96], mybir.dt.float32)
            nc.sync.dma_start(out=wt[:], in_=wpack[:, :])

            # 10 pre-shifted feature windows of x (host-materialized):
            # xs[s][f, m, t] = x at global feature P*128 + f - s, time t
            xs = []
            for s in range(HALO + 1):
                t = xpool.tile([PCHUNK, M, THALF + 2], mybir.dt.float16,
                               name=f"xs{s}", tag=f"xs{s}")
                nc.sync.dma_start(out=t[:], in_=x_local[s])
                xs.append(t)

            chunks = []
            pos = 0
            for tc_len in TCHUNKS:
                chunks.append((pos, tc_len))
                pos += tc_len
            assert pos == THALF
            for (t0, tc_len) in chunks:
                staging = stg.tile([PCHUNK, tc_len * 96], mybir.dt.float16,
                                   name="staging", tag="staging",
                                   padded_shape=[PCHUNK, max(TCHUNKS) * 96])
                st5 = staging.rearrange("p (t l m i) -> p m t l i",
                                        t=tc_len, l=L, m=M, i=K)
                for idx, (i, l) in enumerate(IL):
                    s = i + l
                    il = i * L + l
                    xt = xs[s]
                    x0 = xt[:, :, t0 + 0:t0 + tc_len]
                    x1 = xt[:, :, t0 + 1:t0 + 1 + tc_len]
                    x2 = xt[:, :, t0 + 2:t0 + 2 + tc_len]
                    y0 = work.tile([PCHUNK, M, tc_len], mybir.dt.float16,
                                   name="y0", tag="y0",
                                   padded_shape=[PCHUNK, M, max(TCHUNKS)])
                    t1 = work.tile([PCHUNK, M, tc_len], mybir.dt.float16,
                                   name="t1", tag="t1",
                                   padded_shape=[PCHUNK, M, max(TCHUNKS)])
                    y1 = work.tile([PCHUNK, M, tc_len], mybir.dt.float16,
                                   name="y1", tag="y1",
                                   padded_shape=[PCHUNK, M, max(TCHUNKS)])
                    # ~29% of slabs run as pure DVE chains (no cross-engine
                    # handoffs); the rest as ACT/ACT -> pool -> DVE.
                    dve_own = idx % 7 in (1, 4)
                    a1_dve = (not dve_own) and idx % 8 == 0
                    # y0 = w0*x(t-2) + b
                    if dve_own or a1_dve:
                        nc.vector.tensor_scalar(
                            out=y0[:], in0=x0, scalar1=wt[:, il:il + 1],
                            scalar2=wt[:, 72 + il:73 + il],
                            op0=mybir.AluOpType.mult, op1=mybir.AluOpType.add)
                    else:
                        nc.scalar.activation(
                            out=y0[:], in_=x0,
                            func=mybir.ActivationFunctionType.Identity,
                            scale=wt[:, il:il + 1], bias=wt[:, 72 + il:73 + il])
                    # t1 = w1*x(t-1)
                    if dve_own:
                        nc.vector.tensor_scalar(
                            out=t1[:], in0=x1, scalar1=wt[:, 24 + il:25 + il],
                            scalar2=None, op0=mybir.AluOpType.mult)
                    else:
                        nc.scalar.activation(
                            out=t1[:], in_=x1,
                            func=mybir.ActivationFunctionType.Identity,
                            scale=wt[:, 24 + il:25 + il], bias=0.0)
                    # y1 = y0 + t1
                    eng = nc.vector if dve_own else nc.gpsimd
                    eng.tensor_tensor(
                        out=y1[:], in0=y0[:], in1=t1[:],
                        op=mybir.AluOpType.add)
                    # staging[:, m, t, l, i] = w2*x(t) + y1  (strided write)
                    nc.vector.scalar_tensor_tensor(
                        out=st5[:, :, :, l, i], in0=x2,
                        scalar=wt[:, 48 + il:49 + il], in1=y1[:],
                        op0=mybir.AluOpType.mult, op1=mybir.AluOpType.add)
                # fp16 staging -> fp16 DRAM (host upcasts to f32)
                nc.sync.dma_start(
                    out=out_local[:, t0 * 96:(t0 + tc_len) * 96],
                    in_=staging[:])
            if timing:
                mk = wpool.tile([PCHUNK, 1], mybir.dt.float32, name="mk")
                nc.vector.tensor_copy(out=mk[:], in_=wt[:, 0:1])
                nc.sync.dma_start(out=marker[:, :], in_=mk[:])
    nc.compile()
    return nc


def _build_program_timing():
    return _build_program(timing=True)


def _build_empty_program():
    from concourse import mybir, bacc
    from concourse.tile import TileContext

    nc = bacc.Bacc("TRN2", target_bir_lowering=False, debug=False,
                   num_devices=NCORES)
    din = nc.dram_tensor("dummy_in", (1, 1), mybir.dt.float32,
                         kind="ExternalInput")
    dout = nc.dram_tensor("dummy_out", (1, 1), mybir.dt.float32,
                          kind="ExternalOutput")
    with TileContext(nc) as tc:
        with tc.tile_pool(name="p", bufs=1) as pool:
            t = pool.tile([1, 1], mybir.dt.float32)
            nc.sync.dma_start(out=t[:], in_=din[:, :])
            nc.sync.dma_start(out=dout[:, :], in_=t[:])
    nc.compile()
    return nc


def _prep_inputs(x, conv_w, conv_b):
    """Host-side prep: transpose/pad/cast x, pre-shift weights per core."""
    x = np.asarray(x, dtype=np.float32)
    conv_w = np.asarray(conv_w, dtype=np.float32).reshape(F, K * L, CK)
    conv_b = np.asarray(conv_b, dtype=np.float32).reshape(F, K * L)

    xT = np.transpose(x, (0, 2, 1))  # (M, F, T)
    xTpad = np.zeros((M, F, T + 2), dtype=np.float16)
    xTpad[:, :, 2:] = xT.astype(np.float16)

    in_maps = []
    for core in range(NCORES):
        P, th = divmod(core, 2)
        tsl = xTpad[:, :, th * THALF:th * THALF + THALF + 2]  # (M, F, 514)
        x_loc = np.empty((HALO + 1, PCHUNK, M, THALF + 2), dtype=np.float16)
        for s in range(HALO + 1):
            fidx = (np.arange(P * PCHUNK - s, P * PCHUNK - s + PCHUNK)) % F
            x_loc[s] = tsl[:, fidx].transpose(1, 0, 2)

        wp = np.empty((PCHUNK, 96), dtype=np.float32)
        f_out = np.arange(P * PCHUNK, P * PCHUNK + PCHUNK)
        for i in range(K):
            for l in range(L):
                il = i * L + l
                f_src = (f_out - (i + l)) % F
                wp[:, il] = conv_w[f_src, il, 0]
                wp[:, 24 + il] = conv_w[f_src, il, 1]
                wp[:, 48 + il] = conv_w[f_src, il, 2]
                wp[:, 72 + il] = conv_b[f_src, il]
        in_maps.append({"x_local": x_loc, "wpack": wp})
    return in_maps


def _assemble(results):
    full = np.empty((F, T, L * M, K), dtype=np.float32)
    for core in range(NCORES):
        P, th = divmod(core, 2)
        blk = results[core]["out_local"].astype(np.float32)
        blk = blk.reshape(PCHUNK, THALF, L, M, K)
        blk = blk.transpose(0, 1, 2, 3, 4).reshape(PCHUNK, THALF, L * M, K)
        full[P * PCHUNK:(P + 1) * PCHUNK, th * THALF:(th + 1) * THALF] = blk
    # time mask: out[:, t, l*M+m, i] = 0 for t < i + l
    for l in range(L):
        for i in range(K):
            s = i + l
            if s:
                full[:, :s, l * M:(l + 1) * M, i] = 0.0
    return full


def kernel(x, conv_w, conv_b, _want_trace=False):
    from concourse.bass_utils import run_bass_kernel_spmd

    if "nc" not in _prog_cache:
        _prog_cache["nc"] = _build_program()
    nc = _prog_cache["nc"]

    in_maps = _prep_inputs(x, conv_w, conv_b)
    res = run_bass_kernel_spmd(nc, in_maps, core_ids=list(range(NCORES)),
                               trace=_want_trace)
    out = _assemble(res.results)
    if _want_trace:
        return out, res
    return out



# revision 29
# speedup vs baseline: 1.7631x; 1.7631x over previous
"""Trainium2 Bass kernel for nn_CAConvV2 (grouped causal conv + per-tap
feature roll + time mask, output (F, T, L*M, K)).

Self-contained: hardcodes shapes/sharding for
  x: (4, 1024, 512) f32, conv_w: (12288, 1, 3) f32, conv_b: (12288,) f32
  output: (512, 1024, 12, 8) f32

Sharding: 8 cores = 4 feature chunks (128) x 2 time halves (512).
No cross-core communication.

Design: each core loads ONE unshifted x slice (128, 4, 514) fp16; the
per-(i,l) feature roll is applied at host assembly time (output row
placement), so the device computes the plain grouped conv
  y[g, il, m, t] = b + w0*x(t-2) + w1*x(t-1) + w2*x(t)
for its 128 feature groups. The 24 (i,l) slabs are spread across engine
lanes sized to the cost model:
  'E' (x12): PE diagonal matmuls accumulate the 3 taps in PSUM
             (diag(w_c) built once from an identity), ACT evicts with
             the bias add.
  'M' (x8):  DVE tensor_scalar products (4x fp16 mode) with one product
             on ACT, sums via DVE tensor_tensor (2x).
  'P' (x4):  DVE products, Pool tensor_tensor sums.
All SBUF writes are contiguous fp16; output DMA (12.6 MB/core) is the
roofline resource.
"""

import numpy as np

M, T, F = 4, 1024, 512
K, L, CK = 8, 3, 3
NCORES = 8
PCHUNK = 128  # features per core
THALF = 512   # time steps per core
NSLAB = K * L             # 24 (i,l) slabs
SLAB_FREE = M * THALF     # 2048 elements per partition per slab

# Lane per emission slot; slab il == emission index. 'E' = PE+ACT,
# 'M' = DVE+one ACT product, 'P' = DVE products + Pool adds, 'D' = pure
# DVE. First slab is 'D' (starts as soon as x chunks land); last slabs
# avoid ACT so the drain runs on DVE while ACT finishes evictions.
LANES = ['E', 'P', 'E', 'D', 'E', 'P', 'E', 'M', 'E', 'P', 'E', 'M',
         'E', 'P', 'E', 'M', 'E', 'D', 'E', 'M', 'E', 'D', 'E', 'D']
assert len(LANES) == NSLAB
assert LANES.count('E') == 12 and LANES.count('M') == 4
assert LANES.count('P') == 4 and LANES.count('D') == 4

_prog_cache = {}


def _build_program(timing=False):
    from concourse import mybir, bacc
    from concourse.tile import TileContext

    nc = bacc.Bacc("TRN2", target_bir_lowering=False, debug=False,
                   num_devices=NCORES)
    x_local = nc.dram_tensor("x_local", (PCHUNK, M, THALF + 2),
                             mybir.dt.float16, kind="ExternalInput")
    # wpack columns: [w0 (24) | w1 (24) | w2 (24) | bias (24)]
    wpack = nc.dram_tensor("wpack", (PCHUNK, 96), mybir.dt.float32,
                           kind="ExternalInput")
    ident = nc.dram_tensor("ident", (PCHUNK, PCHUNK), mybir.dt.float16,
                           kind="ExternalInput")
    out_local = nc.dram_tensor("out_local", (PCHUNK, NSLAB * SLAB_FREE),
                               mybir.dt.float16,
                               kind="Internal" if timing else "ExternalOutput")
    if timing:
        marker = nc.dram_tensor("marker", (PCHUNK, 1), mybir.dt.float32,
                                kind="ExternalOutput")

    with TileContext(nc) as tc:
        with tc.tile_pool(name="xp", bufs=1) as xpool, \
             tc.tile_pool(name="wp", bufs=1) as wpool, \
             tc.tile_pool(name="dg", bufs=1) as dgpool, \
             tc.tile_pool(name="work", bufs=5) as work, \
             tc.tile_pool(name="workp", bufs=4) as workp, \
             tc.tile_pool(name="stg", bufs=6) as stg, \
             tc.tile_pool(name="stgp", bufs=4) as stgp, \
             tc.tile_pool(name="ps", bufs=2, space="PSUM") as pp:
            xt = xpool.tile([PCHUNK, M, THALF + 2], mybir.dt.float16,
                            name="xt")
            wt = wpool.tile([PCHUNK, 96], mybir.dt.float32)
            idt = wpool.tile([PCHUNK, PCHUNK], mybir.dt.float16, name="idt")
            warm = wpool.tile([PCHUNK, 1], mybir.dt.float32, name="warm")
            # Warm the ACT function table during input DMA so the first
            # real activation doesn't pay the table load.
            nc.gpsimd.memset(warm[:], 0.0)
            nc.scalar.activation(out=warm[:], in_=warm[:],
                                 func=mybir.ActivationFunctionType.Identity,
                                 scale=1.0, bias=0.0)
            nc.sync.dma_start(out=xt[:, 0:1], in_=x_local[:, 0:1])
            nc.sync.dma_start(out=wt[:], in_=wpack[:, :])
            nc.sync.dma_start(out=xt[:, 1:2], in_=x_local[:, 1:2])
            nc.sync.dma_start(out=idt[:], in_=ident[:, :])
            nc.sync.dma_start(out=xt[:, 2:3], in_=x_local[:, 2:3])
            nc.sync.dma_start(out=xt[:, 3:4], in_=x_local[:, 3:4])

            x0 = xt[:, :, 0:THALF]
            x1 = xt[:, :, 1:1 + THALF]
            x2 = xt[:, :, 2:2 + THALF]

            diags = {}

            def emit_builds(il):
                # Diagonal weight matrices for a PE-lane slab, built from
                # the identity: diag(w_c[:, il]) = I * w.
                for c in range(3):
                    d = dgpool.tile([PCHUNK, PCHUNK], mybir.dt.float16,
                                    name=f"diag{il}_{c}")
                    nc.vector.tensor_scalar(
                        out=d[:], in0=idt[:],
                        scalar1=wt[:, c * 24 + il:c * 24 + il + 1],
                        scalar2=None, op0=mybir.AluOpType.mult)
                    diags[(il, c)] = d

            state = {}

            def emit_stage1(il, lane):
                w0 = wt[:, il:il + 1]
                w1 = wt[:, 24 + il:25 + il]
                w2 = wt[:, 48 + il:49 + il]
                bb = wt[:, 72 + il:73 + il]
                if lane == 'E':
                    psum = pp.tile([PCHUNK, M, THALF], mybir.dt.float32,
                                   name="psum", tag="psum")
                    for m in range(M):
                        for c in range(3):
                            nc.tensor.matmul(
                                out=psum[:, m, :], lhsT=diags[(il, c)][:],
                                rhs=xt[:, m, c:c + THALF],
                                start=(c == 0), stop=(c == 2))
                    state[il] = (psum,)
                else:
                    wk = workp if lane == 'P' else work
                    tg = 'q' if lane == 'P' else 'p'
                    p01 = wk.tile([PCHUNK, M, THALF], mybir.dt.float16,
                                  name="p01", tag=f"{tg}01")
                    p1 = wk.tile([PCHUNK, M, THALF], mybir.dt.float16,
                                 name="p1", tag=f"{tg}1")
                    p2 = wk.tile([PCHUNK, M, THALF], mybir.dt.float16,
                                 name="p2", tag=f"{tg}2")
                    nc.vector.tensor_scalar(
                        out=p01[:], in0=x0, scalar1=w0, scalar2=bb,
                        op0=mybir.AluOpType.mult, op1=mybir.AluOpType.add)
                    if lane == 'M':
                        nc.scalar.activation(
                            out=p1[:], in_=x1,
                            func=mybir.ActivationFunctionType.Identity,
                            scale=w1, bias=0.0)
                    else:
                        nc.vector.tensor_scalar(
                            out=p1[:], in0=x1, scalar1=w1, scalar2=None,
                            op0=mybir.AluOpType.mult)
                    nc.vector.tensor_scalar(
                        out=p2[:], in0=x2, scalar1=w2, scalar2=None,
                        op0=mybir.AluOpType.mult)
                    state[il] = (p01, p1, p2)

            def emit_stage2(il, lane):
                bb = wt[:, 72 + il:73 + il]
                spool = stgp if lane == 'P' else stg
                staging = spool.tile([PCHUNK, M, THALF], mybir.dt.float16,
                                     name="staging",
                                     tag="stagingp" if lane == 'P'
                                     else "staging")
                if lane == 'E':
                    (psum,) = state.pop(il)
                    nc.scalar.activation(
                        out=staging[:], in_=psum[:],
                        func=mybir.ActivationFunctionType.Identity,
                        scale=1.0, bias=bb)
                    dma_eng = nc.sync
                else:
                    p01, p1, p2 = state.pop(il)
                    eng = nc.gpsimd if lane == 'P' else nc.vector
                    eng.tensor_tensor(
                        out=p1[:], in0=p01[:], in1=p1[:],
                        op=mybir.AluOpType.add)
                    eng.tensor_tensor(
                        out=staging[:], in0=p1[:], in1=p2[:],
                        op=mybir.AluOpType.add)
                    dma_eng = nc.gpsimd if lane == 'P' else nc.sync
                # Issue the output DMA from the engine that produced the
                # staging tile: each engine's DMA stream stays in its own
                # completion order, so a slow lane never head-of-line
                # blocks the others' output DMAs.
                dma_eng.dma_start(
                    out=out_local[:, il * SLAB_FREE:(il + 1) * SLAB_FREE],
                    in_=staging[:])

            # --- Prologue: slab 0 (E) and slab 1 (P) emitted per-m so the
            # first output DMAs start as soon as the x chunks land. Diag
            # builds for slab 0/2 interleave with slab 1's product ops in
            # expected-ready order.
            assert LANES[0] == 'E' and LANES[1] == 'P'
            emit_builds(0)
            q01 = workp.tile([PCHUNK, M, THALF], mybir.dt.float16,
                             name="p01", tag="q01")
            q1 = workp.tile([PCHUNK, M, THALF], mybir.dt.float16,
                            name="p1", tag="q1")
            q2 = workp.tile([PCHUNK, M, THALF], mybir.dt.float16,
                            name="p2", tag="q2")
            psum0 = pp.tile([PCHUNK, M, THALF], mybir.dt.float32,
                            name="psum", tag="psum")
            stag0 = stg.tile([PCHUNK, M, THALF], mybir.dt.float16,
                             name="staging", tag="staging")
            for m in range(M):
                for c in range(3):
                    nc.tensor.matmul(
                        out=psum0[:, m, :], lhsT=diags[(0, c)][:],
                        rhs=xt[:, m, c:c + THALF],
                        start=(c == 0), stop=(c == 2))
                nc.vector.tensor_scalar(
                    out=q01[:, m], in0=xt[:, m, 0:THALF],
                    scalar1=wt[:, 1:2], scalar2=wt[:, 73:74],
                    op0=mybir.AluOpType.mult, op1=mybir.AluOpType.add)
                nc.vector.tensor_scalar(
                    out=q1[:, m], in0=xt[:, m, 1:1 + THALF],
                    scalar1=wt[:, 25:26], scalar2=None,
                    op0=mybir.AluOpType.mult)
                nc.vector.tensor_scalar(
                    out=q2[:, m], in0=xt[:, m, 2:2 + THALF],
                    scalar1=wt[:, 49:50], scalar2=None,
                    op0=mybir.AluOpType.mult)
                if m == 0:
                    emit_builds(2)
                # Per-m eviction + quarter DMA of slab 0: output bytes
                # flow while later x chunks are still arriving.
                nc.scalar.activation(
                    out=stag0[:, m], in_=psum0[:, m],
                    func=mybir.ActivationFunctionType.Identity,
                    scale=1.0, bias=wt[:, 72:73])
                nc.sync.dma_start(
                    out=out_local[:, m * THALF:(m + 1) * THALF],
                    in_=stag0[:, m])
            state[1] = (q01, q1, q2)

            # --- Steady phase: depth-2 software pipeline. stage2 of slab
            # w-1 interleaves with stage1 of slab w so in-order engine
            # queues never convoy on a cross-engine dependency; diag
            # builds run one wave ahead of their PE slab.
            for w in range(2, NSLAB + 1):
                if w + 1 < NSLAB and LANES[w + 1] == 'E':
                    emit_builds(w + 1)
                if w < NSLAB:
                    emit_stage1(w, LANES[w])
                emit_stage2(w - 1, LANES[w - 1])
            if timing:
                mk = wpool.tile([PCHUNK, 1], mybir.dt.float32, name="mk")
                nc.vector.tensor_copy(out=mk[:], in_=wt[:, 0:1])
                nc.sync.dma_start(out=marker[:, :], in_=mk[:])
    nc.compile()
    return nc


def _build_program_timing():
    return _build_program(timing=True)


def _build_empty_program():
    from concourse import mybir, bacc
    from concourse.tile import TileContext

    nc = bacc.Bacc("TRN2", target_bir_lowering=False, debug=False,
                   num_devices=NCORES)
    din = nc.dram_tensor("dummy_in", (1, 1), mybir.dt.float32,
                         kind="ExternalInput")
    dout = nc.dram_tensor("dummy_out", (1, 1), mybir.dt.float32,
                          kind="ExternalOutput")
    with TileContext(nc) as tc:
        with tc.tile_pool(name="p", bufs=1) as pool:
            t = pool.tile([1, 1], mybir.dt.float32)
            nc.sync.dma_start(out=t[:], in_=din[:, :])
            nc.sync.dma_start(out=dout[:, :], in_=t[:])
    nc.compile()
    return nc


def _prep_inputs(x, conv_w, conv_b):
    """Host-side prep: transpose/pad/cast x, slice weights per core."""
    x = np.asarray(x, dtype=np.float32)
    conv_w = np.asarray(conv_w, dtype=np.float32).reshape(F, K * L, CK)
    conv_b = np.asarray(conv_b, dtype=np.float32).reshape(F, K * L)

    xT = np.transpose(x, (0, 2, 1))  # (M, F, T)
    xTpad = np.zeros((M, F, T + 2), dtype=np.float16)
    xTpad[:, :, 2:] = xT.astype(np.float16)
    ident = np.eye(PCHUNK, dtype=np.float16)

    in_maps = []
    for core in range(NCORES):
        P, th = divmod(core, 2)
        fsl = slice(P * PCHUNK, (P + 1) * PCHUNK)
        x_loc = np.ascontiguousarray(
            xTpad[:, fsl, th * THALF:th * THALF + THALF + 2]
            .transpose(1, 0, 2))  # (128, M, 514)
        wp = np.concatenate(
            [conv_w[fsl, :, 0], conv_w[fsl, :, 1], conv_w[fsl, :, 2],
             conv_b[fsl, :]], axis=1).astype(np.float32)  # (128, 96)
        in_maps.append({"x_local": x_loc, "wpack": wp, "ident": ident})
    return in_maps


def _assemble(results):
    # Unshifted conv output per (global feature g, time t, il, m).
    y_full = np.empty((F, T, NSLAB, M), dtype=np.float32)
    for core in range(NCORES):
        P, th = divmod(core, 2)
        blk = results[core]["out_local"].astype(np.float32)
        blk = blk.reshape(PCHUNK, NSLAB, M, THALF)
        y_full[P * PCHUNK:(P + 1) * PCHUNK,
               th * THALF:(th + 1) * THALF] = blk.transpose(0, 3, 1, 2)
    # Apply the per-(i,l) feature roll + time mask at assembly:
    # out[f, t, l*M+m, i] = (t >= s) * y_full[(f - s) % F, t, il, m]
    full = np.empty((F, T, L * M, K), dtype=np.float32)
    for i in range(K):
        for l in range(L):
            il = i * L + l
            s = i + l
            rolled = np.roll(y_full[:, :, il, :], s, axis=0)  # (F, T, M)
            full[:, :, l * M:(l + 1) * M, i] = rolled
            if s:
                full[:, :s, l * M:(l + 1) * M, i] = 0.0
    return full


def kernel(x, conv_w, conv_b, _want_trace=False):
    from concourse.bass_utils import run_bass_kernel_spmd

    if "nc" not in _prog_cache:
        _prog_cache["nc"] = _build_program()
    nc = _prog_cache["nc"]

    in_maps = _prep_inputs(x, conv_w, conv_b)
    res = run_bass_kernel_spmd(nc, in_maps, core_ids=list(range(NCORES)),
                               trace=_want_trace)
    out = _assemble(res.results)
    if _want_trace:
        return out, res
    return out


# revision 42
# speedup vs baseline: 1.8132x; 1.0284x over previous
"""Trainium2 Bass kernel for nn_CAConvV2 (grouped causal conv + per-tap
feature roll + time mask, output (F, T, L*M, K)).

Self-contained: hardcodes shapes/sharding for
  x: (4, 1024, 512) f32, conv_w: (12288, 1, 3) f32, conv_b: (12288,) f32
  output: (512, 1024, 12, 8) f32

Sharding: 8 cores = 4 feature chunks (128) x 2 time halves (512).
No cross-core communication.

Design: each core loads ONE unshifted x slice (128, 4, 514) fp16; the
per-(i,l) feature roll is applied at host assembly time (output row
placement), so the device computes the plain grouped conv
  y[g, il, m, t] = b + w0*x(t-2) + w1*x(t-1) + w2*x(t)
for its 128 feature groups. The 24 (i,l) slabs are spread across engine
lanes sized to the cost model:
  'E' (x12): PE diagonal matmuls accumulate the 3 taps in PSUM
             (diag(w_c) built once from an identity), ACT evicts with
             the bias add.
  'M'/'D':   DVE tensor_scalar products (4x fp16 mode), sums via DVE
             tensor_tensor (2x); 'M' puts one product on ACT.
  'P' (x4):  DVE products, Pool tensor_tensor sums.
Emission is software-pipelined (stage2 of slab w-1 interleaves with
stage1 of slab w) so the in-order engine queues never convoy; each
lane's output DMA is issued from the engine that produced its staging
tile (SP / Pool-SWDGE) so a slow lane cannot head-of-line block the
others. The PE warms its pstate on junk matmuls chained into the first
real PSUM; slab 0 runs per-m with quarter DMAs so output bytes flow
while x is still arriving. All SBUF writes are contiguous fp16; output
DMA (12.6 MB/core) is the roofline resource.
"""

import numpy as np

M, T, F = 4, 1024, 512
K, L, CK = 8, 3, 3
NCORES = 8
PCHUNK = 128  # features per core
THALF = 512   # time steps per core
NSLAB = K * L             # 24 (i,l) slabs
SLAB_FREE = M * THALF     # 2048 elements per partition per slab

# Lane per emission slot; slab il == emission index. 'E' = PE+ACT,
# 'M' = DVE+one ACT product, 'P' = DVE products + Pool adds, 'D' = pure
# DVE. Slab 0 is 'D' (starts as soon as x chunks land), slab 1 'P'
# (Pool starts early); the tail avoids ACT so the drain runs on DVE
# while ACT finishes evictions.
LANES = ['D', 'P', 'E', 'M', 'E', 'P', 'E', 'M', 'E', 'P', 'E', 'M',
         'E', 'P', 'E', 'M', 'E', 'E', 'M', 'E', 'E', 'D', 'E', 'D']
assert len(LANES) == NSLAB
assert LANES.count('E') == 12 and LANES.count('M') == 5
assert LANES.count('P') == 4 and LANES.count('D') == 3

_prog_cache = {}


def _build_program(timing=False):
    from concourse import mybir, bacc
    from concourse.tile import TileContext

    nc = bacc.Bacc("TRN2", target_bir_lowering=False, debug=False,
                   num_devices=NCORES)
    x_local = nc.dram_tensor("x_local", (PCHUNK, M, THALF + 2),
                             mybir.dt.float16, kind="ExternalInput")
    # wpack columns: [w0 (24) | w1 (24) | w2 (24) | bias (24)]
    wpack = nc.dram_tensor("wpack", (PCHUNK, 96), mybir.dt.float32,
                           kind="ExternalInput")
    ident = nc.dram_tensor("ident", (PCHUNK, PCHUNK), mybir.dt.float16,
                           kind="ExternalInput")
    out_local = nc.dram_tensor("out_local", (PCHUNK, NSLAB * SLAB_FREE),
                               mybir.dt.float16,
                               kind="Internal" if timing else "ExternalOutput")
    if timing:
        marker = nc.dram_tensor("marker", (PCHUNK, 1), mybir.dt.float32,
                                kind="ExternalOutput")

    with TileContext(nc) as tc:
        with tc.tile_pool(name="xp", bufs=1) as xpool, \
             tc.tile_pool(name="wp", bufs=1) as wpool, \
             tc.tile_pool(name="dg", bufs=1) as dgpool, \
             tc.tile_pool(name="work", bufs=5) as work, \
             tc.tile_pool(name="workp", bufs=4) as workp, \
             tc.tile_pool(name="stg", bufs=6) as stg, \
             tc.tile_pool(name="stgp", bufs=4) as stgp, \
             tc.tile_pool(name="ps", bufs=2, space="PSUM") as pp:
            xt = xpool.tile([PCHUNK, M, THALF + 2], mybir.dt.float16,
                            name="xt")
            wt = wpool.tile([PCHUNK, 96], mybir.dt.float32)
            idt = wpool.tile([PCHUNK, PCHUNK], mybir.dt.float16, name="idt")
            warm = wpool.tile([PCHUNK, 1], mybir.dt.float32, name="warm")
            # Warm the ACT function table during input DMA so the first
            # real activation doesn't pay the table load.
            nc.gpsimd.memset(warm[:], 0.0)
            nc.scalar.activation(out=warm[:], in_=warm[:],
                                 func=mybir.ActivationFunctionType.Identity,
                                 scale=1.0, bias=0.0)
            nc.sync.dma_start(out=xt[:, 0:1], in_=x_local[:, 0:1])
            nc.sync.dma_start(out=wt[:], in_=wpack[:, :])
            nc.sync.dma_start(out=xt[:, 1:2], in_=x_local[:, 1:2])
            nc.sync.dma_start(out=idt[:], in_=ident[:, :])
            nc.sync.dma_start(out=xt[:, 2:3], in_=x_local[:, 2:3])
            nc.sync.dma_start(out=xt[:, 3:4], in_=x_local[:, 3:4])

            x0 = xt[:, :, 0:THALF]
            x1 = xt[:, :, 1:1 + THALF]
            x2 = xt[:, :, 2:2 + THALF]

            diags = {}

            def emit_builds(il, on_act=False):
                # Diagonal weight matrices for a PE-lane slab, built from
                # the identity: diag(w_c[:, il]) = I * w. The first few
                # build on the otherwise-idle ACT engine so neither the
                # DVE prologue nor the PE waits.
                for c in range(3):
                    d = dgpool.tile([PCHUNK, PCHUNK], mybir.dt.float16,
                                    name=f"diag{il}_{c}")
                    if on_act:
                        nc.scalar.activation(
                            out=d[:], in_=idt[:],
                            func=mybir.ActivationFunctionType.Identity,
                            scale=wt[:, c * 24 + il:c * 24 + il + 1],
                            bias=0.0)
                    else:
                        nc.vector.tensor_scalar(
                            out=d[:], in0=idt[:],
                            scalar1=wt[:, c * 24 + il:c * 24 + il + 1],
                            scalar2=None, op0=mybir.AluOpType.mult)
                    diags[(il, c)] = d

            state = {}

            def emit_stage1(il, lane):
                w0 = wt[:, il:il + 1]
                w1 = wt[:, 24 + il:25 + il]
                w2 = wt[:, 48 + il:49 + il]
                bb = wt[:, 72 + il:73 + il]
                if lane == 'E':
                    psum = state.pop(('pre', il), None)
                    if psum is None:
                        psum = pp.tile([PCHUNK, M, THALF], mybir.dt.float32,
                                       name="psum", tag="psum")
                    for m in range(M):
                        for c in range(3):
                            nc.tensor.matmul(
                                out=psum[:, m, :], lhsT=diags[(il, c)][:],
                                rhs=xt[:, m, c:c + THALF],
                                start=(c == 0), stop=(c == 2))
                    state[il] = (psum,)
                else:
                    wk = workp if lane == 'P' else work
                    tg = 'q' if lane == 'P' else 'p'
                    p01 = wk.tile([PCHUNK, M, THALF], mybir.dt.float16,
                                  name="p01", tag=f"{tg}01")
                    p1 = wk.tile([PCHUNK, M, THALF], mybir.dt.float16,
                                 name="p1", tag=f"{tg}1")
                    p2 = wk.tile([PCHUNK, M, THALF], mybir.dt.float16,
                                 name="p2", tag=f"{tg}2")
                    nc.vector.tensor_scalar(
                        out=p01[:], in0=x0, scalar1=w0, scalar2=bb,
                        op0=mybir.AluOpType.mult, op1=mybir.AluOpType.add)
                    if lane == 'M':
                        nc.scalar.activation(
                            out=p1[:], in_=x1,
                            func=mybir.ActivationFunctionType.Identity,
                            scale=w1, bias=0.0)
                    else:
                        nc.vector.tensor_scalar(
                            out=p1[:], in0=x1, scalar1=w1, scalar2=None,
                            op0=mybir.AluOpType.mult)
                    nc.vector.tensor_scalar(
                        out=p2[:], in0=x2, scalar1=w2, scalar2=None,
                        op0=mybir.AluOpType.mult)
                    state[il] = (p01, p1, p2)

            def emit_stage2(il, lane):
                bb = wt[:, 72 + il:73 + il]
                spool = stgp if lane == 'P' else stg
                staging = spool.tile([PCHUNK, M, THALF], mybir.dt.float16,
                                     name="staging",
                                     tag="stagingp" if lane == 'P'
                                     else "staging")
                if lane == 'E':
                    (psum,) = state.pop(il)
                    nc.scalar.activation(
                        out=staging[:], in_=psum[:],
                        func=mybir.ActivationFunctionType.Identity,
                        scale=1.0, bias=bb)
                    dma_eng = nc.sync
                else:
                    p01, p1, p2 = state.pop(il)
                    eng = nc.gpsimd if lane == 'P' else nc.vector
                    if lane == 'P' and il == 1:
                        # First Pool slab: per-half adds so Pool starts on
                        # the early m chunks' products.
                        for h in range(2):
                            sl = slice(2 * h, 2 * h + 2)
                            eng.tensor_tensor(
                                out=p1[:, sl], in0=p01[:, sl],
                                in1=p1[:, sl], op=mybir.AluOpType.add)
                            eng.tensor_tensor(
                                out=staging[:, sl], in0=p1[:, sl],
                                in1=p2[:, sl], op=mybir.AluOpType.add)
                    else:
                        eng.tensor_tensor(
                            out=p1[:], in0=p01[:], in1=p1[:],
                            op=mybir.AluOpType.add)
                        eng.tensor_tensor(
                            out=staging[:], in0=p1[:], in1=p2[:],
                            op=mybir.AluOpType.add)
                    dma_eng = nc.gpsimd if lane == 'P' else nc.sync
                # Issue the output DMA from the engine that produced the
                # staging tile: each engine's DMA stream stays in its own
                # completion order, so a slow lane never head-of-line
                # blocks the others' output DMAs.
                dma_eng.dma_start(
                    out=out_local[:, il * SLAB_FREE:(il + 1) * SLAB_FREE],
                    in_=staging[:])

            # --- Prologue. Slab 0 (pure DVE) runs per-m with quarter
            # DMAs so output bytes flow while x is still arriving; slab 1
            # (P) gets per-m products so Pool starts early; the PE warms
            # its pstate on junk matmuls (WAW-chained into slab 2's psum
            # so the pipeline stays continuously busy into real work).
            assert LANES[0] == 'D' and LANES[1] == 'P'
            jnk = wpool.tile([PCHUNK, PCHUNK], mybir.dt.float16,
                             name="jnk")
            nc.gpsimd.memset(jnk[:], 0.0)
            emit_builds(2, on_act=True)
            emit_builds(4, on_act=True)
            emit_builds(6, on_act=True)
            psum2 = pp.tile([PCHUNK, M, THALF], mybir.dt.float32,
                            name="psum", tag="psum")
            for _ in range(32):
                nc.tensor.matmul(out=psum2[:, 0, 0:128], lhsT=jnk[:],
                                 rhs=jnk[:, 0:128], start=True, stop=True)

            d01 = work.tile([PCHUNK, M, THALF], mybir.dt.float16,
                            name="p01", tag="p01")
            d1 = work.tile([PCHUNK, M, THALF], mybir.dt.float16,
                           name="p1", tag="p1")
            d2 = work.tile([PCHUNK, M, THALF], mybir.dt.float16,
                           name="p2", tag="p2")
            stag0 = stg.tile([PCHUNK, M, THALF], mybir.dt.float16,
                             name="staging", tag="staging")
            q01 = workp.tile([PCHUNK, M, THALF], mybir.dt.float16,
                             name="p01", tag="q01")
            q1 = workp.tile([PCHUNK, M, THALF], mybir.dt.float16,
                            name="p1", tag="q1")
            q2 = workp.tile([PCHUNK, M, THALF], mybir.dt.float16,
                            name="p2", tag="q2")

            def _emit_q_products(m):
                # Slab 1 (P lane) products, one m chunk at a time.
                nc.vector.tensor_scalar(
                    out=q01[:, m], in0=xt[:, m, 0:THALF],
                    scalar1=wt[:, 1:2], scalar2=wt[:, 73:74],
                    op0=mybir.AluOpType.mult, op1=mybir.AluOpType.add)
                nc.vector.tensor_scalar(
                    out=q1[:, m], in0=xt[:, m, 1:1 + THALF],
                    scalar1=wt[:, 25:26], scalar2=None,
                    op0=mybir.AluOpType.mult)
                nc.vector.tensor_scalar(
                    out=q2[:, m], in0=xt[:, m, 2:2 + THALF],
                    scalar1=wt[:, 49:50], scalar2=None,
                    op0=mybir.AluOpType.mult)

            for m in range(M):
                nc.vector.tensor_scalar(
                    out=d01[:, m], in0=xt[:, m, 0:THALF],
                    scalar1=wt[:, 0:1], scalar2=wt[:, 72:73],
                    op0=mybir.AluOpType.mult, op1=mybir.AluOpType.add)
                nc.vector.tensor_scalar(
                    out=d1[:, m], in0=xt[:, m, 1:1 + THALF],
                    scalar1=wt[:, 24:25], scalar2=None,
                    op0=mybir.AluOpType.mult)
                nc.vector.tensor_scalar(
                    out=d2[:, m], in0=xt[:, m, 2:2 + THALF],
                    scalar1=wt[:, 48:49], scalar2=None,
                    op0=mybir.AluOpType.mult)
                nc.vector.tensor_tensor(
                    out=d1[:, m], in0=d01[:, m], in1=d1[:, m],
                    op=mybir.AluOpType.add)
                nc.vector.tensor_tensor(
                    out=stag0[:, m], in0=d1[:, m], in1=d2[:, m],
                    op=mybir.AluOpType.add)
                nc.sync.dma_start(
                    out=out_local[:, m * THALF:(m + 1) * THALF],
                    in_=stag0[:, m])
                _emit_q_products(m)
            state[1] = (q01, q1, q2)
            state[('pre', 2)] = psum2

            # --- Steady phase: depth-2 software pipeline. stage2 of slab
            # w-1 interleaves with stage1 of slab w so in-order engine
            # queues never convoy on a cross-engine dependency; diag
            # builds run one wave ahead of their PE slab.
            for w in range(2, NSLAB + 1):
                if w + 1 < NSLAB and LANES[w + 1] == 'E' and w + 1 > 6:
                    emit_builds(w + 1)
                if w < NSLAB:
                    emit_stage1(w, LANES[w])
                emit_stage2(w - 1, LANES[w - 1])
            if timing:
                mk = wpool.tile([PCHUNK, 1], mybir.dt.float32, name="mk")
                nc.vector.tensor_copy(out=mk[:], in_=wt[:, 0:1])
                nc.sync.dma_start(out=marker[:, :], in_=mk[:])
    nc.compile()
    return nc


def _build_program_timing():
    return _build_program(timing=True)


def _build_empty_program():
    from concourse import mybir, bacc
    from concourse.tile import TileContext

    nc = bacc.Bacc("TRN2", target_bir_lowering=False, debug=False,
                   num_devices=NCORES)
    din = nc.dram_tensor("dummy_in", (1, 1), mybir.dt.float32,
                         kind="ExternalInput")
    dout = nc.dram_tensor("dummy_out", (1, 1), mybir.dt.float32,
                          kind="ExternalOutput")
    with TileContext(nc) as tc:
        with tc.tile_pool(name="p", bufs=1) as pool:
            t = pool.tile([1, 1], mybir.dt.float32)
            nc.sync.dma_start(out=t[:], in_=din[:, :])
            nc.sync.dma_start(out=dout[:, :], in_=t[:])
    nc.compile()
    return nc


def _prep_inputs(x, conv_w, conv_b):
    """Host-side prep: transpose/pad/cast x, slice weights per core."""
    x = np.asarray(x, dtype=np.float32)
    conv_w = np.asarray(conv_w, dtype=np.float32).reshape(F, K * L, CK)
    conv_b = np.asarray(conv_b, dtype=np.float32).reshape(F, K * L)

    xT = np.transpose(x, (0, 2, 1))  # (M, F, T)
    xTpad = np.zeros((M, F, T + 2), dtype=np.float16)
    xTpad[:, :, 2:] = xT.astype(np.float16)
    ident = np.eye(PCHUNK, dtype=np.float16)

    in_maps = []
    for core in range(NCORES):
        P, th = divmod(core, 2)
        fsl = slice(P * PCHUNK, (P + 1) * PCHUNK)
        x_loc = np.ascontiguousarray(
            xTpad[:, fsl, th * THALF:th * THALF + THALF + 2]
            .transpose(1, 0, 2))  # (128, M, 514)
        wp = np.concatenate(
            [conv_w[fsl, :, 0], conv_w[fsl, :, 1], conv_w[fsl, :, 2],
             conv_b[fsl, :]], axis=1).astype(np.float32)  # (128, 96)
        in_maps.append({"x_local": x_loc, "wpack": wp, "ident": ident})
    return in_maps


def _assemble(results):
    # Unshifted conv output per (global feature g, time t, il, m).
    y_full = np.empty((F, T, NSLAB, M), dtype=np.float32)
    for core in range(NCORES):
        P, th = divmod(core, 2)
        blk = results[core]["out_local"].astype(np.float32)
        blk = blk.reshape(PCHUNK, NSLAB, M, THALF)
        y_full[P * PCHUNK:(P + 1) * PCHUNK,
               th * THALF:(th + 1) * THALF] = blk.transpose(0, 3, 1, 2)
    # Apply the per-(i,l) feature roll + time mask at assembly:
    # out[f, t, l*M+m, i] = (t >= s) * y_full[(f - s) % F, t, il, m]
    full = np.empty((F, T, L * M, K), dtype=np.float32)
    for i in range(K):
        for l in range(L):
            il = i * L + l
            s = i + l
            rolled = np.roll(y_full[:, :, il, :], s, axis=0)  # (F, T, M)
            full[:, :, l * M:(l + 1) * M, i] = rolled
            if s:
                full[:, :s, l * M:(l + 1) * M, i] = 0.0
    return full


def kernel(x, conv_w, conv_b, _want_trace=False):
    from concourse.bass_utils import run_bass_kernel_spmd

    if "nc" not in _prog_cache:
        _prog_cache["nc"] = _build_program()
    nc = _prog_cache["nc"]

    in_maps = _prep_inputs(x, conv_w, conv_b)
    res = run_bass_kernel_spmd(nc, in_maps, core_ids=list(range(NCORES)),
                               trace=_want_trace)
    out = _assemble(res.results)
    if _want_trace:
        return out, res
    return out


# revision 52
# speedup vs baseline: 1.8312x; 1.0100x over previous
"""Trainium2 Bass kernel for nn_CAConvV2 (grouped causal conv + per-tap
feature roll + time mask, output (F, T, L*M, K)).

Self-contained: hardcodes shapes/sharding for
  x: (4, 1024, 512) f32, conv_w: (12288, 1, 3) f32, conv_b: (12288,) f32
  output: (512, 1024, 12, 8) f32

Sharding: 8 cores = 4 feature chunks (128) x 2 time halves (512).
No cross-core communication.

Design: each core loads ONE unshifted x slice (128, 4, 514) fp16; the
per-(i,l) feature roll is applied at host assembly time (output row
placement), so the device computes the plain grouped conv
  y[g, il, m, t] = b + w0*x(t-2) + w1*x(t-1) + w2*x(t)
for its 128 feature groups. The 24 (i,l) slabs are spread across engine
lanes sized to the cost model:
  'E' (x12): PE diagonal matmuls accumulate the 3 taps in PSUM
             (diag(w_c) built once from an identity), ACT evicts with
             the bias add.
  'M'/'D':   DVE tensor_scalar products (4x fp16 mode), sums via DVE
             tensor_tensor (2x); 'M' puts one product on ACT.
  'P' (x4):  DVE products, Pool tensor_tensor sums.
Emission is software-pipelined (stage2 of slab w-1 interleaves with
stage1 of slab w) so the in-order engine queues never convoy; each
lane's output DMA is issued from the engine that produced its staging
tile (SP / Pool-SWDGE) so a slow lane cannot head-of-line block the
others. The PE warms its pstate on junk matmuls chained into the first
real PSUM; slab 0 runs per-m with quarter DMAs so output bytes flow
while x is still arriving. All SBUF writes are contiguous fp16; output
DMA (12.6 MB/core) is the roofline resource.
"""

import numpy as np

M, T, F = 4, 1024, 512
K, L, CK = 8, 3, 3
NCORES = 8
PCHUNK = 128  # features per core
THALF = 512   # time steps per core
NSLAB = K * L             # 24 (i,l) slabs
SLAB_FREE = M * THALF     # 2048 elements per partition per slab

# Lane per emission slot; slab il == emission index. 'E' = PE+ACT,
# 'M' = DVE+one ACT product, 'P' = DVE products + Pool adds, 'D' = pure
# DVE. Slab 0 is 'D' (starts as soon as x chunks land), slab 1 'P'
# (Pool starts early); the tail avoids ACT so the drain runs on DVE
# while ACT finishes evictions.
LANES = ['D', 'P', 'E', 'M', 'E', 'P', 'E', 'M', 'E', 'P', 'E', 'M',
         'E', 'P', 'E', 'M', 'E', 'E', 'M', 'E', 'E', 'D', 'E', 'D']
assert len(LANES) == NSLAB
assert LANES.count('E') == 12 and LANES.count('M') == 5
assert LANES.count('P') == 4 and LANES.count('D') == 3

_prog_cache = {}


def _build_program(timing=False):
    from concourse import mybir, bacc
    from concourse.tile import TileContext

    nc = bacc.Bacc("TRN2", target_bir_lowering=False, debug=False,
                   num_devices=NCORES)
    x_local = nc.dram_tensor("x_local", (PCHUNK, M, THALF + 2),
                             mybir.dt.float16, kind="ExternalInput")
    # wpack columns: [w0 (24) | w1 (24) | w2 (24) | bias (24)]
    wpack = nc.dram_tensor("wpack", (PCHUNK, 96), mybir.dt.float32,
                           kind="ExternalInput")
    ident = nc.dram_tensor("ident", (PCHUNK, PCHUNK), mybir.dt.float16,
                           kind="ExternalInput")
    out_local = nc.dram_tensor("out_local", (PCHUNK, NSLAB * SLAB_FREE),
                               mybir.dt.float16,
                               kind="Internal" if timing else "ExternalOutput")
    if timing:
        marker = nc.dram_tensor("marker", (PCHUNK, 1), mybir.dt.float32,
                                kind="ExternalOutput")

    with TileContext(nc) as tc:
        with tc.tile_pool(name="xp", bufs=1) as xpool, \
             tc.tile_pool(name="wp", bufs=1) as wpool, \
             tc.tile_pool(name="dg", bufs=1) as dgpool, \
             tc.tile_pool(name="work", bufs=5) as work, \
             tc.tile_pool(name="workp", bufs=4) as workp, \
             tc.tile_pool(name="stg", bufs=6) as stg, \
             tc.tile_pool(name="stgp", bufs=4) as stgp, \
             tc.tile_pool(name="ps", bufs=2, space="PSUM") as pp:
            xt = xpool.tile([PCHUNK, M, THALF + 2], mybir.dt.float16,
                            name="xt")
            wt = wpool.tile([PCHUNK, 96], mybir.dt.float32)
            idt = wpool.tile([PCHUNK, PCHUNK], mybir.dt.float16, name="idt")
            warm = wpool.tile([PCHUNK, 1], mybir.dt.float32, name="warm")
            # Warm the ACT function table during input DMA so the first
            # real activation doesn't pay the table load.
            nc.gpsimd.memset(warm[:], 0.0)
            nc.scalar.activation(out=warm[:], in_=warm[:],
                                 func=mybir.ActivationFunctionType.Identity,
                                 scale=1.0, bias=0.0)
            nc.sync.dma_start(out=xt[:, 0:1], in_=x_local[:, 0:1])
            nc.sync.dma_start(out=wt[:], in_=wpack[:, :])
            nc.sync.dma_start(out=xt[:, 1:2], in_=x_local[:, 1:2])
            nc.sync.dma_start(out=idt[:], in_=ident[:, :])
            nc.sync.dma_start(out=xt[:, 2:3], in_=x_local[:, 2:3])
            nc.sync.dma_start(out=xt[:, 3:4], in_=x_local[:, 3:4])

            x0 = xt[:, :, 0:THALF]
            x1 = xt[:, :, 1:1 + THALF]
            x2 = xt[:, :, 2:2 + THALF]

            diags = {}

            def emit_builds(il, on_act=False):
                # Diagonal weight matrices for a PE-lane slab, built from
                # the identity: diag(w_c[:, il]) = I * w. The first few
                # build on the otherwise-idle ACT engine so neither the
                # DVE prologue nor the PE waits.
                for c in range(3):
                    d = dgpool.tile([PCHUNK, PCHUNK], mybir.dt.float16,
                                    name=f"diag{il}_{c}")
                    if on_act:
                        nc.scalar.activation(
                            out=d[:], in_=idt[:],
                            func=mybir.ActivationFunctionType.Identity,
                            scale=wt[:, c * 24 + il:c * 24 + il + 1],
                            bias=0.0)
                    else:
                        nc.vector.tensor_scalar(
                            out=d[:], in0=idt[:],
                            scalar1=wt[:, c * 24 + il:c * 24 + il + 1],
                            scalar2=None, op0=mybir.AluOpType.mult)
                    diags[(il, c)] = d

            state = {}

            def emit_stage1(il, lane):
                w0 = wt[:, il:il + 1]
                w1 = wt[:, 24 + il:25 + il]
                w2 = wt[:, 48 + il:49 + il]
                bb = wt[:, 72 + il:73 + il]
                if lane == 'E':
                    psum = state.pop(('pre', il), None)
                    if psum is None:
                        psum = pp.tile([PCHUNK, M, THALF], mybir.dt.float32,
                                       name="psum", tag="psum")
                    for m in range(M):
                        for c in range(3):
                            nc.tensor.matmul(
                                out=psum[:, m, :], lhsT=diags[(il, c)][:],
                                rhs=xt[:, m, c:c + THALF],
                                start=(c == 0), stop=(c == 2))
                    state[il] = (psum,)
                else:
                    wk = workp if lane == 'P' else work
                    tg = 'q' if lane == 'P' else 'p'
                    p01 = wk.tile([PCHUNK, M, THALF], mybir.dt.float16,
                                  name="p01", tag=f"{tg}01")
                    p1 = wk.tile([PCHUNK, M, THALF], mybir.dt.float16,
                                 name="p1", tag=f"{tg}1")
                    p2 = wk.tile([PCHUNK, M, THALF], mybir.dt.float16,
                                 name="p2", tag=f"{tg}2")
                    nc.vector.tensor_scalar(
                        out=p01[:], in0=x0, scalar1=w0, scalar2=bb,
                        op0=mybir.AluOpType.mult, op1=mybir.AluOpType.add)
                    if lane == 'M':
                        nc.scalar.activation(
                            out=p1[:], in_=x1,
                            func=mybir.ActivationFunctionType.Identity,
                            scale=w1, bias=0.0)
                    else:
                        nc.vector.tensor_scalar(
                            out=p1[:], in0=x1, scalar1=w1, scalar2=None,
                            op0=mybir.AluOpType.mult)
                    nc.vector.tensor_scalar(
                        out=p2[:], in0=x2, scalar1=w2, scalar2=None,
                        op0=mybir.AluOpType.mult)
                    state[il] = (p01, p1, p2)

            def emit_stage2(il, lane):
                bb = wt[:, 72 + il:73 + il]
                spool = stgp if lane == 'P' else stg
                staging = spool.tile([PCHUNK, M, THALF], mybir.dt.float16,
                                     name="staging",
                                     tag="stagingp" if lane == 'P'
                                     else "staging")
                if lane == 'E':
                    (psum,) = state.pop(il)
                    if il == 2:
                        # First PE slab evicts per half with half DMAs to
                        # keep the DMA engines fed during ramp-up.
                        for h in range(2):
                            sl = slice(2 * h, 2 * h + 2)
                            nc.scalar.activation(
                                out=staging[:, sl], in_=psum[:, sl],
                                func=mybir.ActivationFunctionType.Identity,
                                scale=1.0, bias=bb)
                            nc.sync.dma_start(
                                out=out_local[:,
                                              il * SLAB_FREE
                                              + 2 * h * THALF:
                                              il * SLAB_FREE
                                              + (2 * h + 2) * THALF],
                                in_=staging[:, sl])
                        return
                    nc.scalar.activation(
                        out=staging[:], in_=psum[:],
                        func=mybir.ActivationFunctionType.Identity,
                        scale=1.0, bias=bb)
                    dma_eng = nc.sync
                else:
                    p01, p1, p2 = state.pop(il)
                    eng = nc.gpsimd if lane == 'P' else nc.vector
                    if lane == 'P' and il == 1:
                        # First Pool slab: per-half adds so Pool starts on
                        # the early m chunks' products.
                        for h in range(2):
                            sl = slice(2 * h, 2 * h + 2)
                            eng.tensor_tensor(
                                out=p1[:, sl], in0=p01[:, sl],
                                in1=p1[:, sl], op=mybir.AluOpType.add)
                            eng.tensor_tensor(
                                out=staging[:, sl], in0=p1[:, sl],
                                in1=p2[:, sl], op=mybir.AluOpType.add)
                    else:
                        eng.tensor_tensor(
                            out=p1[:], in0=p01[:], in1=p1[:],
                            op=mybir.AluOpType.add)
                        eng.tensor_tensor(
                            out=staging[:], in0=p1[:], in1=p2[:],
                            op=mybir.AluOpType.add)
                    dma_eng = nc.gpsimd if lane == 'P' else nc.sync
                # Issue the output DMA from the engine that produced the
                # staging tile: each engine's DMA stream stays in its own
                # completion order, so a slow lane never head-of-line
                # blocks the others' output DMAs.
                dma_eng.dma_start(
                    out=out_local[:, il * SLAB_FREE:(il + 1) * SLAB_FREE],
                    in_=staging[:])

            # --- Prologue. Slab 0 (pure DVE) runs per-m with quarter
            # DMAs so output bytes flow while x is still arriving; slab 1
            # (P) gets per-m products so Pool starts early; the PE warms
            # its pstate on junk matmuls (WAW-chained into slab 2's psum
            # so the pipeline stays continuously busy into real work).
            assert LANES[0] == 'D' and LANES[1] == 'P'
            jnk = wpool.tile([PCHUNK, PCHUNK], mybir.dt.float16,
                             name="jnk")
            nc.gpsimd.memset(jnk[:], 0.0)
            emit_builds(2, on_act=True)
            emit_builds(4, on_act=True)
            emit_builds(6, on_act=True)
            psum2 = pp.tile([PCHUNK, M, THALF], mybir.dt.float32,
                            name="psum", tag="psum")
            for _ in range(32):
                nc.tensor.matmul(out=psum2[:, 0, 0:128], lhsT=jnk[:],
                                 rhs=jnk[:, 0:128], start=True, stop=True)

            d01 = work.tile([PCHUNK, M, THALF], mybir.dt.float16,
                            name="p01", tag="p01")
            d1 = work.tile([PCHUNK, M, THALF], mybir.dt.float16,
                           name="p1", tag="p1")
            d2 = work.tile([PCHUNK, M, THALF], mybir.dt.float16,
                           name="p2", tag="p2")
            stag0 = stg.tile([PCHUNK, M, THALF], mybir.dt.float16,
                             name="staging", tag="staging")
            q01 = workp.tile([PCHUNK, M, THALF], mybir.dt.float16,
                             name="p01", tag="q01")
            q1 = workp.tile([PCHUNK, M, THALF], mybir.dt.float16,
                            name="p1", tag="q1")
            q2 = workp.tile([PCHUNK, M, THALF], mybir.dt.float16,
                            name="p2", tag="q2")

            def _emit_q_products(m):
                # Slab 1 (P lane) products, one m chunk at a time.
                nc.vector.tensor_scalar(
                    out=q01[:, m], in0=xt[:, m, 0:THALF],
                    scalar1=wt[:, 1:2], scalar2=wt[:, 73:74],
                    op0=mybir.AluOpType.mult, op1=mybir.AluOpType.add)
                nc.vector.tensor_scalar(
                    out=q1[:, m], in0=xt[:, m, 1:1 + THALF],
                    scalar1=wt[:, 25:26], scalar2=None,
                    op0=mybir.AluOpType.mult)
                nc.vector.tensor_scalar(
                    out=q2[:, m], in0=xt[:, m, 2:2 + THALF],
                    scalar1=wt[:, 49:50], scalar2=None,
                    op0=mybir.AluOpType.mult)

            for m in range(M):
                nc.vector.tensor_scalar(
                    out=d01[:, m], in0=xt[:, m, 0:THALF],
                    scalar1=wt[:, 0:1], scalar2=wt[:, 72:73],
                    op0=mybir.AluOpType.mult, op1=mybir.AluOpType.add)
                nc.vector.tensor_scalar(
                    out=d1[:, m], in0=xt[:, m, 1:1 + THALF],
                    scalar1=wt[:, 24:25], scalar2=None,
                    op0=mybir.AluOpType.mult)
                nc.vector.tensor_scalar(
                    out=d2[:, m], in0=xt[:, m, 2:2 + THALF],
                    scalar1=wt[:, 48:49], scalar2=None,
                    op0=mybir.AluOpType.mult)
                nc.vector.tensor_tensor(
                    out=d1[:, m], in0=d01[:, m], in1=d1[:, m],
                    op=mybir.AluOpType.add)
                nc.vector.tensor_tensor(
                    out=stag0[:, m], in0=d1[:, m], in1=d2[:, m],
                    op=mybir.AluOpType.add)
                nc.sync.dma_start(
                    out=out_local[:, m * THALF:(m + 1) * THALF],
                    in_=stag0[:, m])
                _emit_q_products(m)
            state[1] = (q01, q1, q2)
            state[('pre', 2)] = psum2

            # --- Steady phase: depth-2 software pipeline. stage2 of slab
            # w-1 interleaves with stage1 of slab w so in-order engine
            # queues never convoy on a cross-engine dependency; diag
            # builds run one wave ahead of their PE slab.
            for w in range(2, NSLAB + 1):
                if w + 1 < NSLAB and LANES[w + 1] == 'E' and w + 1 > 6:
                    emit_builds(w + 1)
                if w < NSLAB:
                    emit_stage1(w, LANES[w])
                emit_stage2(w - 1, LANES[w - 1])
            if timing:
                mk = wpool.tile([PCHUNK, 1], mybir.dt.float32, name="mk")
                nc.vector.tensor_copy(out=mk[:], in_=wt[:, 0:1])
                nc.sync.dma_start(out=marker[:, :], in_=mk[:])
    nc.compile()
    return nc


def _build_program_timing():
    return _build_program(timing=True)


def _build_empty_program():
    from concourse import mybir, bacc
    from concourse.tile import TileContext

    nc = bacc.Bacc("TRN2", target_bir_lowering=False, debug=False,
                   num_devices=NCORES)
    din = nc.dram_tensor("dummy_in", (1, 1), mybir.dt.float32,
                         kind="ExternalInput")
    dout = nc.dram_tensor("dummy_out", (1, 1), mybir.dt.float32,
                          kind="ExternalOutput")
    with TileContext(nc) as tc:
        with tc.tile_pool(name="p", bufs=1) as pool:
            t = pool.tile([1, 1], mybir.dt.float32)
            nc.sync.dma_start(out=t[:], in_=din[:, :])
            nc.sync.dma_start(out=dout[:, :], in_=t[:])
    nc.compile()
    return nc


def _prep_inputs(x, conv_w, conv_b):
    """Host-side prep: transpose/pad/cast x, slice weights per core."""
    x = np.asarray(x, dtype=np.float32)
    conv_w = np.asarray(conv_w, dtype=np.float32).reshape(F, K * L, CK)
    conv_b = np.asarray(conv_b, dtype=np.float32).reshape(F, K * L)

    xT = np.transpose(x, (0, 2, 1))  # (M, F, T)
    xTpad = np.zeros((M, F, T + 2), dtype=np.float16)
    xTpad[:, :, 2:] = xT.astype(np.float16)
    ident = np.eye(PCHUNK, dtype=np.float16)

    in_maps = []
    for core in range(NCORES):
        P, th = divmod(core, 2)
        fsl = slice(P * PCHUNK, (P + 1) * PCHUNK)
        x_loc = np.ascontiguousarray(
            xTpad[:, fsl, th * THALF:th * THALF + THALF + 2]
            .transpose(1, 0, 2))  # (128, M, 514)
        wp = np.concatenate(
            [conv_w[fsl, :, 0], conv_w[fsl, :, 1], conv_w[fsl, :, 2],
             conv_b[fsl, :]], axis=1).astype(np.float32)  # (128, 96)
        in_maps.append({"x_local": x_loc, "wpack": wp, "ident": ident})
    return in_maps


def _assemble(results):
    # Unshifted conv output per (global feature g, time t, il, m).
    y_full = np.empty((F, T, NSLAB, M), dtype=np.float32)
    for core in range(NCORES):
        P, th = divmod(core, 2)
        blk = results[core]["out_local"].astype(np.float32)
        blk = blk.reshape(PCHUNK, NSLAB, M, THALF)
        y_full[P * PCHUNK:(P + 1) * PCHUNK,
               th * THALF:(th + 1) * THALF] = blk.transpose(0, 3, 1, 2)
    # Apply the per-(i,l) feature roll + time mask at assembly:
    # out[f, t, l*M+m, i] = (t >= s) * y_full[(f - s) % F, t, il, m]
    full = np.empty((F, T, L * M, K), dtype=np.float32)
    for i in range(K):
        for l in range(L):
            il = i * L + l
            s = i + l
            rolled = np.roll(y_full[:, :, il, :], s, axis=0)  # (F, T, M)
            full[:, :, l * M:(l + 1) * M, i] = rolled
            if s:
                full[:, :s, l * M:(l + 1) * M, i] = 0.0
    return full


def kernel(x, conv_w, conv_b, _want_trace=False):
    from concourse.bass_utils import run_bass_kernel_spmd

    if "nc" not in _prog_cache:
        _prog_cache["nc"] = _build_program()
    nc = _prog_cache["nc"]

    in_maps = _prep_inputs(x, conv_w, conv_b)
    res = run_bass_kernel_spmd(nc, in_maps, core_ids=list(range(NCORES)),
                               trace=_want_trace)
    out = _assemble(res.results)
    if _want_trace:
        return out, res
    return out
